# revision 55
# baseline (speedup 1.0000x reference)
"""Multi-head attention (B=2, S=2048, E=1024, H=16, D=64) on 8 NeuronCores.

Sharding: core c -> batch b = c//4, head group g = c%4 (4 heads, 256 channels).
Each core: Q/K/V projections for its channels, attention for its 4 heads,
and a partial output projection (sum over its 256 concat channels).
Host sums the 4 partials per batch and adds the output bias.

On-chip layout (per core):
  - x*T inputs pre-transposed on host: [E, S] so SBUF holds [e_part, s_free].
  - Q, K in [channel, seq] layout -> scores computed transposed S^T[k, q];
    head pairs sit at partition bases 0/64 so the d=64 score matmuls pack
    into disjoint PE row groups and run concurrently.
  - softmax: exp on ScalarE (no max subtraction needed; |scores| < ~3),
    denominator via a ones-column appended to V in the P@V matmul,
    normalization applied to the 64x512 attention output (not the SxS
    matrix).
  - All matmul operands bf16, fp32 PSUM accumulation.
  - PSUM: 3 x 2-bank "stage" slots (scores + all projection psums) and
    2 x 1-bank softmax accumulators = 8 banks exactly.
"""

import numpy as np
import ml_dtypes

B, S, E, H, D = 2, 2048, 1024, 16, 64
N_CORES = 8
HPC = 4          # heads per core
CH = HPC * D     # 256 channels per core
QB = 512         # q block (matmul moving free dim)
NQ = S // QB     # 4
NK = S // 128    # 16 k partition chunks
NE = E // 128    # 8 e chunks

_CACHE = {}


def _build():
    import concourse.tile as tile
    from concourse import bacc, mybir

    bf16 = mybir.dt.bfloat16
    f32 = mybir.dt.float32
    Exp = mybir.ActivationFunctionType.Exp
    Ident = mybir.ActivationFunctionType.Identity

    nc = bacc.Bacc("TRN2", target_bir_lowering=False, debug=False,
                   num_devices=N_CORES)

    xqT_d = nc.dram_tensor("xqT", [E, S], bf16, kind="ExternalInput").ap()
    xkT_d = nc.dram_tensor("xkT", [E, S], bf16, kind="ExternalInput").ap()
    xvT_d = nc.dram_tensor("xvT", [E, S], bf16, kind="ExternalInput").ap()
    wqT_d = nc.dram_tensor("wqT", [E, CH], bf16, kind="ExternalInput").ap()
    wkT_d = nc.dram_tensor("wkT", [E, CH], bf16, kind="ExternalInput").ap()
    wvT_d = nc.dram_tensor("wvT", [E, CH], bf16, kind="ExternalInput").ap()
    woT_d = nc.dram_tensor("woT", [CH, E], bf16, kind="ExternalInput").ap()
    bq_d = nc.dram_tensor("bq", [CH], f32, kind="ExternalInput").ap()
    bk_d = nc.dram_tensor("bk", [CH], f32, kind="ExternalInput").ap()
    bv_d = nc.dram_tensor("bv", [CH], f32, kind="ExternalInput").ap()
    outT_d = nc.dram_tensor("outT", [E, S], bf16, kind="ExternalOutput").ap()

    import concourse.bass as bass

    with tile.TileContext(nc) as tc:
        with (
            tc.tile_pool(name="singles", bufs=1) as singles,
            tc.tile_pool(name="exps", bufs=8) as exps_pool,
            tc.tile_pool(name="small", bufs=5) as small_pool,
            tc.tile_pool(name="outp", bufs=5) as out_pool,
            tc.tile_pool(name="psum", bufs=3, space="PSUM") as psum_pool,
        ):
            # ---- persistent SBUF tiles ----
            xq_sb = singles.tile([128, NE, S], bf16)
            xk_sb = singles.tile([128, NE, S], bf16)
            xv_sb = singles.tile([128, NE, S], bf16)
            wq_sb = singles.tile([128, NE, CH], bf16)
            wk_sb = singles.tile([128, NE, CH], bf16)
            wv_sb = singles.tile([128, NE, CH], bf16)
            wo_sb = singles.tile([128, 2, E], bf16)
            bq_sb = singles.tile([128, 2], f32)
            bk_sb = singles.tile([128, 2], f32)
            bvb_sb = singles.tile([128, CH], f32)
            q_sb = singles.tile([128, 2, S], bf16)
            k_sb = singles.tile([128, 2, S], bf16)
            v_sb = singles.tile([128, NK, HPC, D + 1], bf16)
            a0_sb = singles.tile([128, S], bf16)
            a1_sb = singles.tile([128, S], bf16)

            # ---- load weights / biases / inputs ----
            # each phase's weights land just before their X stream
            nc.sync.dma_start(out=wk_sb[:],
                              in_=wkT_d.rearrange("(c p) j -> p c j", p=128))
            nc.sync.dma_start(out=bk_sb[:],
                              in_=bk_d.rearrange("(j p) -> p j", p=128))
            for ec in range(NE):
                nc.sync.dma_start(out=xk_sb[:, ec, :],
                                  in_=xkT_d[ec * 128:(ec + 1) * 128, :])
            nc.sync.dma_start(out=wq_sb[:],
                              in_=wqT_d.rearrange("(c p) j -> p c j", p=128))
            nc.sync.dma_start(out=bq_sb[:],
                              in_=bq_d.rearrange("(j p) -> p j", p=128))
            for ec in range(NE):
                nc.sync.dma_start(out=xq_sb[:, ec, :],
                                  in_=xqT_d[ec * 128:(ec + 1) * 128, :])
            nc.sync.dma_start(out=wv_sb[:],
                              in_=wvT_d.rearrange("(c p) j -> p c j", p=128))
            bv_bcast = bass.AP(tensor=bv_d.tensor, offset=bv_d.offset,
                               ap=[[0, 128]] + bv_d.ap)
            nc.sync.dma_start(out=bvb_sb[:], in_=bv_bcast)
            for ec in range(NE):
                nc.sync.dma_start(out=xv_sb[:, ec, :],
                                  in_=xvT_d[ec * 128:(ec + 1) * 128, :])
            nc.sync.dma_start(out=wo_sb[:],
                              in_=woT_d.rearrange("(i p) e -> p i e", p=128))

            nc.gpsimd.memset(v_sb[:, :, :, D:D + 1], 1.0)



            # ---- K / Q projections: out layout [channel, seq] ----
            # (bias add on ScalarE, which is idle until attention starts)
            # the acc0/acc1 PSUM banks are idle until attention starts; use
            # them as extra in-flight accumulation groups during projections
            def proj_tag(i):
                return (f"acc{i}", 1) if i < 2 else ("stage", None)

            for (w_sb, x_sb, b_sb, dst) in (
                (wk_sb, xk_sb, bk_sb, k_sb),
                (wq_sb, xq_sb, bq_sb, q_sb),
            ):
                for jb in range(2):
                    for qs in range(NQ):
                        tag, tb = proj_tag(jb * NQ + qs)
                        ps = psum_pool.tile([128, QB], f32, tag=tag, bufs=tb,
                                            name=f"pp{jb}{qs}")
                        for ec in range(NE):
                            nc.tensor.matmul(
                                ps[:],
                                lhsT=w_sb[:, ec, jb * 128:(jb + 1) * 128],
                                rhs=x_sb[:, ec, qs * QB:(qs + 1) * QB],
                                start=(ec == 0), stop=(ec == NE - 1))
                        nc.vector.tensor_add(
                            dst[:, jb, qs * QB:(qs + 1) * QB], ps[:],
                            b_sb[:, jb:jb + 1].to_broadcast([128, QB]))

            # ---- V projection: out layout [seq, head, d] (+ ones column) ----
            for sb in range(NK):
                tag, tb = proj_tag(sb)
                ps = psum_pool.tile([128, CH], f32, tag=tag, bufs=tb,
                                    name=f"pv{sb}")
                for ec in range(NE):
                    nc.tensor.matmul(
                        ps[:],
                        lhsT=xv_sb[:, ec, sb * 128:(sb + 1) * 128],
                        rhs=wv_sb[:, ec, :],
                        start=(ec == 0), stop=(ec == NE - 1))
                for h in range(HPC):
                    nc.vector.tensor_add(
                        v_sb[:, sb, h, 0:D],
                        ps[:, h * D:(h + 1) * D],
                        bvb_sb[:, h * D:(h + 1) * D])

            # ---- attention + output projection, per q block ----
            # Oproj for block qs is emitted after attention for block qs+1 so
            # the PE has score/PV matmuls to run while the softmax-normalize
            # chain (DVE recip -> Pool bcast -> DVE mul -> DMA) for qs drains.
            def oproj(qs, eb):
                ps = psum_pool.tile([128, QB], f32, tag="stage",
                                    name=f"po{qs}{eb}")
                for ib in range(2):
                    nc.tensor.matmul(
                        ps[:],
                        lhsT=wo_sb[:, ib, eb * 128:(eb + 1) * 128],
                        rhs=(a0_sb if ib == 0 else a1_sb)[:, qs * QB:(qs + 1) * QB],
                        start=(ib == 0), stop=(ib == 1))
                ot = out_pool.tile([128, QB], bf16, tag="out",
                                   name=f"ot{qs}{eb}")
                if qs == NQ - 1 and eb % 2 == 0:
                    # tail: exps are done, ScalarE is idle — alternate the
                    # copies ACT/DVE so neither engine paces the drain
                    nc.scalar.copy(ot[:], ps[:])
                else:
                    nc.vector.tensor_copy(ot[:], ps[:])
                nc.sync.dma_start(
                    out=outT_d[eb * 128:(eb + 1) * 128,
                               qs * QB:(qs + 1) * QB],
                    in_=ot[:])

            for qs in range(NQ):
                for pair in range(2):
                    acc = [
                        psum_pool.tile([D + 1, QB], f32, tag=f"acc{hh}",
                                       bufs=1, name=f"acc{hh}_{pair}_{qs}")
                        for hh in range(2)
                    ]
                    # PV lags L groups behind scores/exp: PE stays fed
                    # with score matmuls while v_sb / the pair's acc bank
                    # become available (PE executes strictly in order)
                    LAG = 2
                    exq = []

                    def pv(grp):
                        exA, exB = exq[grp % (LAG + 2)]
                        for hh in range(2):
                            h = 2 * pair + hh
                            for c in range(2):
                                kc = grp * 2 + c
                                nc.tensor.matmul(
                                    acc[hh][:],
                                    lhsT=v_sb[:, kc, h, :],
                                    rhs=(exA if hh == 0 else exB)[:, c, :],
                                    start=(kc == 0), stop=(kc == NK - 1))

                    for grp in range(NK // 2):
                        stg = []
                        ex = []
                        for hh in range(2):
                            h = 2 * pair + hh
                            base = 64 * (h % 2)
                            jb = h // 2
                            st = psum_pool.tile(
                                [128, 2, QB], f32, tag="stage",
                                name=f"st{pair}{qs}{grp}{hh}")
                            stg.append(st)
                            for c in range(2):
                                kc = grp * 2 + c
                                nc.tensor.matmul(
                                    st[:, c, :],
                                    lhsT=k_sb[base:base + 64, jb,
                                              kc * 128:(kc + 1) * 128],
                                    rhs=q_sb[base:base + 64, jb,
                                             qs * QB:(qs + 1) * QB],
                                    start=True, stop=True)
                        for hh in range(2):
                            e_t = exps_pool.tile([128, 2, QB], bf16, tag="exp",
                                                 name=f"ex{pair}{qs}{grp}{hh}")
                            ex.append(e_t)
                            nc.scalar.activation(e_t[:], stg[hh][:], Exp,
                                                 scale=0.125)
                        if len(exq) < LAG + 2:
                            exq.append(ex)
                        else:
                            exq[grp % (LAG + 2)] = ex
                        if grp >= LAG:
                            pv(grp - LAG)
                        # spread previous block's output projection through
                        # this block's attention groups (covers DVE copies)
                        if qs > 0 and grp % 2 == 0:
                            oproj(qs - 1, pair * 4 + grp // 2)
                    for grp in range(NK // 2 - LAG, NK // 2):
                        pv(grp)
                    # normalize by softmax denominator (row D of acc)
                    for hh in range(2):
                        h = 2 * pair + hh
                        base = 64 * (h % 2)
                        a_dst = a0_sb if h // 2 == 0 else a1_sb
                        recip = small_pool.tile([1, QB], f32, tag="recip",
                                                name=f"rc{pair}{qs}{hh}")
                        nc.vector.reciprocal(recip[:], acc[hh][D:D + 1, :])
                        recip_b = small_pool.tile([64, QB], f32, tag="recipb",
                                                  name=f"rb{pair}{qs}{hh}")
                        nc.gpsimd.partition_broadcast(recip_b[:], recip[:])
                        if base == 0:
                            # partitions line up: write a_sb directly
                            nc.vector.tensor_mul(
                                a_dst[0:64, qs * QB:(qs + 1) * QB],
                                acc[hh][0:D, :], recip_b[:])
                        else:
                            # DVE can't shift partitions; bounce via DMA
                            anorm = small_pool.tile([64, QB], bf16,
                                                    tag="anorm",
                                                    name=f"an{pair}{qs}{hh}")
                            nc.vector.tensor_mul(anorm[:], acc[hh][0:D, :],
                                                 recip_b[:])
                            nc.sync.dma_start(
                                out=a_dst[base:base + 64,
                                          qs * QB:(qs + 1) * QB],
                                in_=anorm[:])

            # bridge the final normalize chain with throwaway matmuls so the
            # PE clock gate stays open (idle >3.4us re-throttles to 1.2GHz
            # and the last oproj groups would run cold)
            for wu in range(12):
                wups = psum_pool.tile([128, QB], f32, tag="stage",
                                      name=f"wu{wu}")
                nc.tensor.matmul(wups[:], lhsT=wo_sb[:, 0, 0:128],
                                 rhs=wo_sb[:, 1, 0:QB], start=True, stop=True)

            for eb in range(NE):
                oproj(NQ - 1, eb)

    nc.compile()
    return nc


def _get_nc():
    if "nc" not in _CACHE:
        _CACHE["nc"] = _build()
    return _CACHE["nc"]


def _get_runner():
    """Cached jitted SPMD executable (mirrors bass2jax.run_bass_via_pjrt's
    multi-core branch, hoisted so repeat calls hit the jit cache)."""
    if "runner" in _CACHE:
        return _CACHE["runner"]
    import jax
    from jax.experimental.shard_map import shard_map
    from jax.sharding import Mesh, PartitionSpec
    from concourse import mybir
    from concourse.bass2jax import (_bass_exec_p, install_neuronx_cc_hook,
                                    partition_id_tensor)

    nc = _get_nc()
    install_neuronx_cc_hook()
    pname = nc.partition_id_tensor.name if nc.partition_id_tensor else None
    in_names, out_names, out_avals, out_shapes = [], [], [], []
    for alloc in nc.m.functions[0].allocations:
        if not isinstance(alloc, mybir.MemoryLocationSet):
            continue
        name = alloc.memorylocations[0].name
        if alloc.kind == "ExternalInput":
            if name != pname:
                in_names.append(name)
        elif alloc.kind == "ExternalOutput":
            shape = tuple(alloc.tensor_shape)
            dtype = mybir.dt.np(alloc.dtype)
            out_names.append(name)
            out_avals.append(jax.core.ShapedArray(shape, dtype))
            out_shapes.append((shape, dtype))
    n_params = len(in_names)
    n_outs = len(out_avals)
    all_in = in_names + out_names + ([pname] if pname else [])
    donate = tuple(range(n_params, n_params + n_outs))

    def _body(*args):
        operands = list(args)
        if pname is not None:
            operands.append(partition_id_tensor())
        return tuple(_bass_exec_p.bind(
            *operands, out_avals=tuple(out_avals), in_names=tuple(all_in),
            out_names=tuple(out_names), lowering_input_output_aliases=(),
            sim_require_finite=True, sim_require_nnan=True, nc=nc))

    devices = jax.devices()[:N_CORES]
    mesh = Mesh(np.asarray(devices), ("core",))
    sharded = jax.jit(
        shard_map(_body, mesh=mesh,
                  in_specs=(PartitionSpec("core"),) * (n_params + n_outs),
                  out_specs=(PartitionSpec("core"),) * n_outs,
                  check_rep=False),
        donate_argnums=donate, keep_unused=True)
    _CACHE["runner"] = (sharded, in_names, out_names, out_shapes)
    return _CACHE["runner"]


def _run_spmd(in_maps):
    """Run the compiled program on cores 0..7; returns {name: np.ndarray
    of shape [n_cores, *shape]} without re-tracing on repeat calls."""
    sharded, in_names, out_names, out_shapes = _get_runner()
    concat_in = [
        np.concatenate([np.asarray(m[name]) for m in in_maps], axis=0)
        for name in in_names
    ]
    concat_zeros = [
        np.zeros((N_CORES * shape[0], *shape[1:]), dtype)
        for shape, dtype in out_shapes
    ]
    out_arrs = sharded(*concat_in, *concat_zeros)
    return {
        name: np.asarray(out_arrs[i]).reshape(
            N_CORES, *out_shapes[i][0])
        for i, name in enumerate(out_names)
    }


def kernel(query, key, value, Wq, bq, Wk, bk, Wv, bv, Wo, bo):
    bf = ml_dtypes.bfloat16

    # per-batch transposed inputs (shared by the 4 cores of each batch)
    xT = {}
    for b in range(B):
        xT[b] = (
            np.ascontiguousarray(query[b].T).astype(bf),
            np.ascontiguousarray(key[b].T).astype(bf),
            np.ascontiguousarray(value[b].T).astype(bf),
        )
    # per-head-group weights
    wg = {}
    for g in range(HPC):
        r = slice(g * CH, (g + 1) * CH)
        wg[g] = dict(
            wqT=np.ascontiguousarray(Wq[r].T).astype(bf),
            wkT=np.ascontiguousarray(Wk[r].T).astype(bf),
            wvT=np.ascontiguousarray(Wv[r].T).astype(bf),
            woT=np.ascontiguousarray(Wo[:, r].T).astype(bf),
            bq=np.ascontiguousarray(bq[r]).astype(np.float32),
            bk=np.ascontiguousarray(bk[r]).astype(np.float32),
            bv=np.ascontiguousarray(bv[r]).astype(np.float32),
        )

    in_maps = []
    for c in range(N_CORES):
        b, g = c // HPC, c % HPC
        m = dict(xqT=xT[b][0], xkT=xT[b][1], xvT=xT[b][2])
        m.update(wg[g])
        in_maps.append(m)

    outs = _run_spmd(in_maps)["outT"]  # [8, E, S] bf16

    out = np.empty((B, S, E), np.float32)
    for b in range(B):
        acc = outs[b * HPC].astype(np.float32)
        for g in range(1, HPC):
            acc += outs[b * HPC + g].astype(np.float32)
        out[b] = acc.T + bo[None, :]
    return out


# revision 58
# speedup vs baseline: 1.0118x; 1.0118x over previous
"""Multi-head attention (B=2, S=2048, E=1024, H=16, D=64) on 8 NeuronCores.

Sharding: core c -> batch b = c//4, head group g = c%4 (4 heads, 256 channels).
Each core: Q/K/V projections for its channels, attention for its 4 heads,
and a partial output projection (sum over its 256 concat channels).
Host sums the 4 partials per batch and adds the output bias.

On-chip layout (per core):
  - x*T inputs pre-transposed on host: [E, S] so SBUF holds [e_part, s_free].
  - Q, K in [channel, seq] layout -> scores computed transposed S^T[k, q];
    head pairs sit at partition bases 0/64 so the d=64 score matmuls pack
    into disjoint PE row groups and run concurrently.
  - softmax: exp on ScalarE (no max subtraction needed; |scores| < ~3),
    denominator via a ones-column appended to V in the P@V matmul,
    normalization applied to the 64x512 attention output (not the SxS
    matrix).
  - All matmul operands bf16, fp32 PSUM accumulation.
  - PSUM: 3 x 2-bank "stage" slots (scores + all projection psums) and
    2 x 1-bank softmax accumulators = 8 banks exactly.
"""

import numpy as np
import ml_dtypes

B, S, E, H, D = 2, 2048, 1024, 16, 64
N_CORES = 8
HPC = 4          # heads per core
CH = HPC * D     # 256 channels per core
QB = 512         # q block (matmul moving free dim)
NQ = S // QB     # 4
NK = S // 128    # 16 k partition chunks
NE = E // 128    # 8 e chunks

_CACHE = {}


def _build():
    import concourse.tile as tile
    from concourse import bacc, mybir

    bf16 = mybir.dt.bfloat16
    f32 = mybir.dt.float32
    Exp = mybir.ActivationFunctionType.Exp
    Ident = mybir.ActivationFunctionType.Identity

    nc = bacc.Bacc("TRN2", target_bir_lowering=False, debug=False,
                   num_devices=N_CORES)

    xqT_d = nc.dram_tensor("xqT", [E, S], bf16, kind="ExternalInput").ap()
    xkT_d = nc.dram_tensor("xkT", [E, S], bf16, kind="ExternalInput").ap()
    xvT_d = nc.dram_tensor("xvT", [E, S], bf16, kind="ExternalInput").ap()
    wqT_d = nc.dram_tensor("wqT", [E, CH], bf16, kind="ExternalInput").ap()
    wkT_d = nc.dram_tensor("wkT", [E, CH], bf16, kind="ExternalInput").ap()
    wvT_d = nc.dram_tensor("wvT", [E, CH], bf16, kind="ExternalInput").ap()
    woT_d = nc.dram_tensor("woT", [CH, E], bf16, kind="ExternalInput").ap()
    bq_d = nc.dram_tensor("bq", [CH], f32, kind="ExternalInput").ap()
    bk_d = nc.dram_tensor("bk", [CH], f32, kind="ExternalInput").ap()
    bv_d = nc.dram_tensor("bv", [CH], f32, kind="ExternalInput").ap()
    outT_d = nc.dram_tensor("outT", [E, S], bf16, kind="ExternalOutput").ap()

    import concourse.bass as bass

    with tile.TileContext(nc) as tc:
        with (
            tc.tile_pool(name="singles", bufs=1) as singles,
            tc.tile_pool(name="exps", bufs=8) as exps_pool,
            tc.tile_pool(name="small", bufs=5) as small_pool,
            tc.tile_pool(name="outp", bufs=5) as out_pool,
            tc.tile_pool(name="psum", bufs=3, space="PSUM") as psum_pool,
        ):
            # ---- persistent SBUF tiles ----
            xq_sb = singles.tile([128, NE, S], bf16)
            xk_sb = singles.tile([128, NE, S], bf16)
            xv_sb = singles.tile([128, NE, S], bf16)
            wq_sb = singles.tile([128, NE, CH], bf16)
            wk_sb = singles.tile([128, NE, CH], bf16)
            wv_sb = singles.tile([128, NE, CH], bf16)
            wo_sb = singles.tile([128, 2, E], bf16)
            bq_sb = singles.tile([128, 2], f32)
            bk_sb = singles.tile([128, 2], f32)
            bvb_sb = singles.tile([128, CH], f32)
            q_sb = singles.tile([128, 2, S], bf16)
            k_sb = singles.tile([128, 2, S], bf16)
            v_sb = singles.tile([128, NK, HPC, D + 1], bf16)
            a0_sb = singles.tile([128, S], bf16)
            a1_sb = singles.tile([128, S], bf16)

            # ---- load weights / biases / inputs ----
            # each phase's weights land just before their X stream
            nc.sync.dma_start(out=wk_sb[:],
                              in_=wkT_d.rearrange("(c p) j -> p c j", p=128))
            nc.sync.dma_start(out=bk_sb[:],
                              in_=bk_d.rearrange("(j p) -> p j", p=128))
            for ec in range(NE):
                nc.sync.dma_start(out=xk_sb[:, ec, :],
                                  in_=xkT_d[ec * 128:(ec + 1) * 128, :])
            nc.sync.dma_start(out=wq_sb[:],
                              in_=wqT_d.rearrange("(c p) j -> p c j", p=128))
            nc.sync.dma_start(out=bq_sb[:],
                              in_=bq_d.rearrange("(j p) -> p j", p=128))
            for ec in range(NE):
                nc.sync.dma_start(out=xq_sb[:, ec, :],
                                  in_=xqT_d[ec * 128:(ec + 1) * 128, :])
            nc.sync.dma_start(out=wv_sb[:],
                              in_=wvT_d.rearrange("(c p) j -> p c j", p=128))
            bv_bcast = bass.AP(tensor=bv_d.tensor, offset=bv_d.offset,
                               ap=[[0, 128]] + bv_d.ap)
            nc.sync.dma_start(out=bvb_sb[:], in_=bv_bcast)
            for ec in range(NE):
                nc.sync.dma_start(out=xv_sb[:, ec, :],
                                  in_=xvT_d[ec * 128:(ec + 1) * 128, :])
            nc.sync.dma_start(out=wo_sb[:],
                              in_=woT_d.rearrange("(i p) e -> p i e", p=128))

            nc.gpsimd.memset(v_sb[:, :, :, D:D + 1], 1.0)



            # ---- K / Q projections: out layout [channel, seq] ----
            # (bias add on ScalarE, which is idle until attention starts)
            # the acc0/acc1 PSUM banks are idle until attention starts; use
            # them as extra in-flight accumulation groups during projections
            def proj_tag(i):
                return (f"acc{i}", 1) if i < 2 else ("stage", None)

            for (w_sb, x_sb, b_sb, dst) in (
                (wk_sb, xk_sb, bk_sb, k_sb),
                (wq_sb, xq_sb, bq_sb, q_sb),
            ):
                for jb in range(2):
                    for qs in range(NQ):
                        tag, tb = proj_tag(jb * NQ + qs)
                        ps = psum_pool.tile([128, QB], f32, tag=tag, bufs=tb,
                                            name=f"pp{jb}{qs}")
                        for ec in range(NE):
                            nc.tensor.matmul(
                                ps[:],
                                lhsT=w_sb[:, ec, jb * 128:(jb + 1) * 128],
                                rhs=x_sb[:, ec, qs * QB:(qs + 1) * QB],
                                start=(ec == 0), stop=(ec == NE - 1))
                        nc.vector.tensor_add(
                            dst[:, jb, qs * QB:(qs + 1) * QB], ps[:],
                            b_sb[:, jb:jb + 1].to_broadcast([128, QB]))

            # ---- V projection: out layout [seq, head, d] (+ ones column) ----
            for sb in range(NK):
                tag, tb = proj_tag(sb)
                ps = psum_pool.tile([128, CH], f32, tag=tag, bufs=tb,
                                    name=f"pv{sb}")
                for ec in range(NE):
                    nc.tensor.matmul(
                        ps[:],
                        lhsT=xv_sb[:, ec, sb * 128:(sb + 1) * 128],
                        rhs=wv_sb[:, ec, :],
                        start=(ec == 0), stop=(ec == NE - 1))
                for h in range(HPC):
                    nc.vector.tensor_add(
                        v_sb[:, sb, h, 0:D],
                        ps[:, h * D:(h + 1) * D],
                        bvb_sb[:, h * D:(h + 1) * D])

            # ---- attention + output projection, per q block ----
            # Oproj for block qs is emitted after attention for block qs+1 so
            # the PE has score/PV matmuls to run while the softmax-normalize
            # chain (DVE recip -> Pool bcast -> DVE mul -> DMA) for qs drains.
            def oproj(qs, eb):
                ps = psum_pool.tile([128, QB], f32, tag="stage",
                                    name=f"po{qs}{eb}")
                for ib in range(2):
                    nc.tensor.matmul(
                        ps[:],
                        lhsT=wo_sb[:, ib, eb * 128:(eb + 1) * 128],
                        rhs=(a0_sb if ib == 0 else a1_sb)[:, qs * QB:(qs + 1) * QB],
                        start=(ib == 0), stop=(ib == 1))
                ot = out_pool.tile([128, QB], bf16, tag="out",
                                   name=f"ot{qs}{eb}")
                if qs == NQ - 1 and eb % 2 == 0:
                    # tail: exps are done, ScalarE is idle — alternate the
                    # copies ACT/DVE so neither engine paces the drain
                    nc.scalar.copy(ot[:], ps[:])
                else:
                    nc.vector.tensor_copy(ot[:], ps[:])
                nc.sync.dma_start(
                    out=outT_d[eb * 128:(eb + 1) * 128,
                               qs * QB:(qs + 1) * QB],
                    in_=ot[:])

            for qs in range(NQ):
                for pair in range(2):
                    acc = [
                        psum_pool.tile([D + 1, QB], f32, tag=f"acc{hh}",
                                       bufs=1, name=f"acc{hh}_{pair}_{qs}")
                        for hh in range(2)
                    ]
                    # PV lags L groups behind scores/exp: PE stays fed
                    # with score matmuls while v_sb / the pair's acc bank
                    # become available (PE executes strictly in order)
                    LAG = 2
                    exq = []

                    def pv(grp):
                        exA, exB = exq[grp % (LAG + 2)]
                        for hh in range(2):
                            h = 2 * pair + hh
                            for c in range(2):
                                kc = grp * 2 + c
                                nc.tensor.matmul(
                                    acc[hh][:],
                                    lhsT=v_sb[:, kc, h, :],
                                    rhs=(exA if hh == 0 else exB)[:, c, :],
                                    start=(kc == 0), stop=(kc == NK - 1))

                    for grp in range(NK // 2):
                        stg = []
                        ex = []
                        for hh in range(2):
                            h = 2 * pair + hh
                            base = 64 * (h % 2)
                            jb = h // 2
                            st = psum_pool.tile(
                                [128, 2, QB], f32, tag="stage",
                                name=f"st{pair}{qs}{grp}{hh}")
                            stg.append(st)
                            for c in range(2):
                                kc = grp * 2 + c
                                nc.tensor.matmul(
                                    st[:, c, :],
                                    lhsT=k_sb[base:base + 64, jb,
                                              kc * 128:(kc + 1) * 128],
                                    rhs=q_sb[base:base + 64, jb,
                                             qs * QB:(qs + 1) * QB],
                                    start=True, stop=True)
                        for hh in range(2):
                            e_t = exps_pool.tile([128, 2, QB], bf16, tag="exp",
                                                 name=f"ex{pair}{qs}{grp}{hh}")
                            ex.append(e_t)
                            nc.scalar.activation(e_t[:], stg[hh][:], Exp,
                                                 scale=0.125)
                        if len(exq) < LAG + 2:
                            exq.append(ex)
                        else:
                            exq[grp % (LAG + 2)] = ex
                        if grp >= LAG:
                            pv(grp - LAG)
                        # spread previous block's output projection through
                        # this block's attention groups (covers DVE copies)
                        if qs > 0 and grp % 2 == 1:
                            oproj(qs - 1, pair * 4 + grp // 2)
                    for grp in range(NK // 2 - LAG, NK // 2):
                        pv(grp)
                    # normalize by softmax denominator (row D of acc)
                    for hh in range(2):
                        h = 2 * pair + hh
                        base = 64 * (h % 2)
                        a_dst = a0_sb if h // 2 == 0 else a1_sb
                        recip = small_pool.tile([1, QB], f32, tag="recip",
                                                name=f"rc{pair}{qs}{hh}")
                        nc.vector.reciprocal(recip[:], acc[hh][D:D + 1, :])
                        recip_b = small_pool.tile([64, QB], f32, tag="recipb",
                                                  name=f"rb{pair}{qs}{hh}")
                        nc.gpsimd.partition_broadcast(recip_b[:], recip[:])
                        if base == 0:
                            # partitions line up: write a_sb directly
                            nc.vector.tensor_mul(
                                a_dst[0:64, qs * QB:(qs + 1) * QB],
                                acc[hh][0:D, :], recip_b[:])
                        else:
                            # DVE can't shift partitions; bounce via DMA
                            anorm = small_pool.tile([64, QB], bf16,
                                                    tag="anorm",
                                                    name=f"an{pair}{qs}{hh}")
                            nc.vector.tensor_mul(anorm[:], acc[hh][0:D, :],
                                                 recip_b[:])
                            nc.sync.dma_start(
                                out=a_dst[base:base + 64,
                                          qs * QB:(qs + 1) * QB],
                                in_=anorm[:])

            # bridge the final normalize chain with throwaway matmuls so the
            # PE clock gate stays open (idle >3.4us re-throttles to 1.2GHz
            # and the last oproj groups would run cold)
            for wu in range(12):
                wups = psum_pool.tile([128, QB], f32, tag="stage",
                                      name=f"wu{wu}")
                nc.tensor.matmul(wups[:], lhsT=wo_sb[:, 0, 0:128],
                                 rhs=wo_sb[:, 1, 0:QB], start=True, stop=True)

            for eb in range(NE):
                oproj(NQ - 1, eb)

    nc.compile()
    return nc


def _get_nc():
    if "nc" not in _CACHE:
        _CACHE["nc"] = _build()
    return _CACHE["nc"]


def _get_runner():
    """Cached jitted SPMD executable (mirrors bass2jax.run_bass_via_pjrt's
    multi-core branch, hoisted so repeat calls hit the jit cache)."""
    if "runner" in _CACHE:
        return _CACHE["runner"]
    import jax
    from jax.experimental.shard_map import shard_map
    from jax.sharding import Mesh, PartitionSpec
    from concourse import mybir
    from concourse.bass2jax import (_bass_exec_p, install_neuronx_cc_hook,
                                    partition_id_tensor)

    nc = _get_nc()
    install_neuronx_cc_hook()
    pname = nc.partition_id_tensor.name if nc.partition_id_tensor else None
    in_names, out_names, out_avals, out_shapes = [], [], [], []
    for alloc in nc.m.functions[0].allocations:
        if not isinstance(alloc, mybir.MemoryLocationSet):
            continue
        name = alloc.memorylocations[0].name
        if alloc.kind == "ExternalInput":
            if name != pname:
                in_names.append(name)
        elif alloc.kind == "ExternalOutput":
            shape = tuple(alloc.tensor_shape)
            dtype = mybir.dt.np(alloc.dtype)
            out_names.append(name)
            out_avals.append(jax.core.ShapedArray(shape, dtype))
            out_shapes.append((shape, dtype))
    n_params = len(in_names)
    n_outs = len(out_avals)
    all_in = in_names + out_names + ([pname] if pname else [])
    donate = tuple(range(n_params, n_params + n_outs))

    def _body(*args):
        operands = list(args)
        if pname is not None:
            operands.append(partition_id_tensor())
        return tuple(_bass_exec_p.bind(
            *operands, out_avals=tuple(out_avals), in_names=tuple(all_in),
            out_names=tuple(out_names), lowering_input_output_aliases=(),
            sim_require_finite=True, sim_require_nnan=True, nc=nc))

    devices = jax.devices()[:N_CORES]
    mesh = Mesh(np.asarray(devices), ("core",))
    sharded = jax.jit(
        shard_map(_body, mesh=mesh,
                  in_specs=(PartitionSpec("core"),) * (n_params + n_outs),
                  out_specs=(PartitionSpec("core"),) * n_outs,
                  check_rep=False),
        donate_argnums=donate, keep_unused=True)
    _CACHE["runner"] = (sharded, in_names, out_names, out_shapes)
    return _CACHE["runner"]


def _run_spmd(in_maps):
    """Run the compiled program on cores 0..7; returns {name: np.ndarray
    of shape [n_cores, *shape]} without re-tracing on repeat calls."""
    sharded, in_names, out_names, out_shapes = _get_runner()
    concat_in = [
        np.concatenate([np.asarray(m[name]) for m in in_maps], axis=0)
        for name in in_names
    ]
    concat_zeros = [
        np.zeros((N_CORES * shape[0], *shape[1:]), dtype)
        for shape, dtype in out_shapes
    ]
    out_arrs = sharded(*concat_in, *concat_zeros)
    return {
        name: np.asarray(out_arrs[i]).reshape(
            N_CORES, *out_shapes[i][0])
        for i, name in enumerate(out_names)
    }


def kernel(query, key, value, Wq, bq, Wk, bk, Wv, bv, Wo, bo):
    bf = ml_dtypes.bfloat16

    # per-batch transposed inputs (shared by the 4 cores of each batch)
    xT = {}
    for b in range(B):
        xT[b] = (
            np.ascontiguousarray(query[b].T).astype(bf),
            np.ascontiguousarray(key[b].T).astype(bf),
            np.ascontiguousarray(value[b].T).astype(bf),
        )
    # per-head-group weights
    wg = {}
    for g in range(HPC):
        r = slice(g * CH, (g + 1) * CH)
        wg[g] = dict(
            wqT=np.ascontiguousarray(Wq[r].T).astype(bf),
            wkT=np.ascontiguousarray(Wk[r].T).astype(bf),
            wvT=np.ascontiguousarray(Wv[r].T).astype(bf),
            woT=np.ascontiguousarray(Wo[:, r].T).astype(bf),
            bq=np.ascontiguousarray(bq[r]).astype(np.float32),
            bk=np.ascontiguousarray(bk[r]).astype(np.float32),
            bv=np.ascontiguousarray(bv[r]).astype(np.float32),
        )

    in_maps = []
    for c in range(N_CORES):
        b, g = c // HPC, c % HPC
        m = dict(xqT=xT[b][0], xkT=xT[b][1], xvT=xT[b][2])
        m.update(wg[g])
        in_maps.append(m)

    outs = _run_spmd(in_maps)["outT"]  # [8, E, S] bf16

    out = np.empty((B, S, E), np.float32)
    for b in range(B):
        acc = outs[b * HPC].astype(np.float32)
        for g in range(1, HPC):
            acc += outs[b * HPC + g].astype(np.float32)
        out[b] = acc.T + bo[None, :]
    return out
